# revision 1
# baseline (speedup 1.0000x reference)
"""AnticipatorRNN Trainium2 kernel: conv encoder + 2-layer LSTM + dense head.

Data-parallel over batch B=16 across 8 NeuronCores (2 sequences/core).
All matmuls in bf16 (f32 psum accumulate). No collectives.
"""

import os
import numpy as np
import ml_dtypes

import concourse.bass as bass
import concourse.mybir as mybir
from concourse import bacc
import concourse.tile as tile

F32 = mybir.dt.float32
BF16 = mybir.dt.bfloat16
AF = mybir.ActivationFunctionType

B, T, H, W, C = 16, 64, 64, 64, 3
NCORES = 8
BP = B // NCORES          # sequences per core = 2
F = BP * T                # frames per core = 128
O1, O2, O3, O4 = 31, 14, 6, 2
N1, N2, N3, N4 = O1 * O1, O2 * O2, O3 * O3, O4 * O4   # 961, 196, 36, 4

_CACHE = {}
TAPS = [(a, b) for a in range(4) for b in range(4)]


def _build_graph(phases=None):
    if phases is None:
        phases = set((os.environ.get("KPHASES")
                      or "conv12,conv3,conv4,xw1,lstm,head").split(","))
    nc = bacc.Bacc(None, target_bir_lowering=False)

    ic1p = nc.declare_dram_parameter("ic1", [48, F, N1], BF16, isOutput=False)
    actp = nc.declare_dram_parameter("actT", [2, F], BF16, isOutput=False)
    w1p = nc.declare_dram_parameter("w1p", [48, 32], BF16, isOutput=False)
    b1p = nc.declare_dram_parameter("b1p", [128, 1], F32, isOutput=False)
    w2p = nc.declare_dram_parameter("w2p", [32, 16, 64], BF16, isOutput=False)
    b2p = nc.declare_dram_parameter("b2p", [128, 1], F32, isOutput=False)
    w3p = nc.declare_dram_parameter("w3p", [64, 16, 128], BF16, isOutput=False)
    b3p = nc.declare_dram_parameter("b3p", [128, 1], F32, isOutput=False)
    w4p = nc.declare_dram_parameter("w4p", [128, 16, 256], BF16, isOutput=False)
    b4p = nc.declare_dram_parameter("b4p", [128, 2], F32, isOutput=False)
    wx1p = nc.declare_dram_parameter("wx1p", [128, 8, 2048], BF16, isOutput=False)
    wactp = nc.declare_dram_parameter("wactp", [2, 2048], BF16, isOutput=False)
    wh1p = nc.declare_dram_parameter("wh1p", [128, 4, 2048], BF16, isOutput=False)
    bl1p = nc.declare_dram_parameter("bl1p", [128, 16], F32, isOutput=False)
    wl2p = nc.declare_dram_parameter("wl2p", [128, 6, 1024], BF16, isOutput=False)
    bl2p = nc.declare_dram_parameter("bl2p", [128, 16], F32, isOutput=False)
    wd1p = nc.declare_dram_parameter("wd1p", [128, 2, 128], BF16, isOutput=False)
    bd1p = nc.declare_dram_parameter("bd1p", [128, 1], F32, isOutput=False)
    wd2p = nc.declare_dram_parameter("wd2p", [128, 1], BF16, isOutput=False)
    bd2p = nc.declare_dram_parameter("bd2p", [1, 1], F32, isOutput=False)
    outp = nc.declare_dram_parameter("out", [1, F], F32, isOutput=True)

    with tile.TileContext(nc) as tc:
        with (
            tc.tile_pool(name="const", bufs=1) as const,
            tc.tile_pool(name="ic", bufs=4) as icpool,
            tc.tile_pool(name="x2", bufs=4) as x2pool,
            tc.tile_pool(name="big", bufs=1) as big,
            tc.tile_pool(name="ps", bufs=2, space="PSUM") as pspool,
            tc.tile_pool(name="pss", bufs=2, space="PSUM") as psspool,
            tc.tile_pool(name="st", bufs=3) as st,
            tc.tile_pool(name="sc", bufs=4) as sc,
        ):
            # ---- load weights/constants into SBUF ----
            w1s = const.tile([48, 32], BF16)
            nc.sync.dma_start(out=w1s[:, :], in_=w1p[:, :])
            b1s = const.tile([128, 1], F32)
            nc.sync.dma_start(out=b1s[:, :], in_=b1p[:, :])
            w2s = const.tile([128, 16, 64], BF16)
            for j in range(4):
                nc.sync.dma_start(out=w2s[32 * j:32 * j + 32, :, :], in_=w2p[:, :, :])
            b2s = const.tile([128, 1], F32)
            nc.sync.dma_start(out=b2s[:, :], in_=b2p[:, :])
            w3s = const.tile([128, 16, 128], BF16)
            for h in range(2):
                nc.sync.dma_start(out=w3s[64 * h:64 * h + 64, :, :], in_=w3p[:, :, :])
            b3s = const.tile([128, 1], F32)
            nc.sync.dma_start(out=b3s[:, :], in_=b3p[:, :])
            w4s = const.tile([128, 16, 256], BF16)
            nc.sync.dma_start(out=w4s[:, :, :], in_=w4p[:, :, :])
            b4s = const.tile([128, 2], F32)
            nc.sync.dma_start(out=b4s[:, :], in_=b4p[:, :])
            wx1s = const.tile([128, 8, 2048], BF16)
            nc.sync.dma_start(out=wx1s[:, :, :], in_=wx1p[:, :, :])
            wacts = const.tile([2, 2048], BF16)
            nc.sync.dma_start(out=wacts[:, :], in_=wactp[:, :])
            wh1s = const.tile([128, 4, 2048], BF16)
            nc.sync.dma_start(out=wh1s[:, :, :], in_=wh1p[:, :, :])
            bl1s = const.tile([128, 16], F32)
            nc.sync.dma_start(out=bl1s[:, :], in_=bl1p[:, :])
            wl2s = const.tile([128, 6, 1024], BF16)
            nc.sync.dma_start(out=wl2s[:, :, :], in_=wl2p[:, :, :])
            bl2s = const.tile([128, 16], F32)
            nc.sync.dma_start(out=bl2s[:, :], in_=bl2p[:, :])
            wd1s = const.tile([128, 2, 128], BF16)
            nc.sync.dma_start(out=wd1s[:, :, :], in_=wd1p[:, :, :])
            bd1s = const.tile([128, 1], F32)
            nc.sync.dma_start(out=bd1s[:, :], in_=bd1p[:, :])
            wd2s = const.tile([128, 1], BF16)
            nc.sync.dma_start(out=wd2s[:, :], in_=wd2p[:, :])
            bd2s = const.tile([1, 1], F32)
            nc.sync.dma_start(out=bd2s[:, :], in_=bd2p[:, :])
            acts = const.tile([2, F], BF16)
            nc.sync.dma_start(out=acts[:, :], in_=actp[:, :])

            # persistent activations
            x3s = big.tile([128, 64, N2], BF16)     # [(half,ch64), pair, 196]
            x4s = big.tile([128, F, N3], BF16)      # [ch128, f, 36]
            encs = big.tile([128, 8, F], BF16)      # [row, ktile, (b,t)]
            xw1s = big.tile([128, 16, F], F32)      # [gate-row, mtile, (b,t)]
            h2hist = big.tile([128, 2, F], BF16)    # [row, ktile, (b,t)]

            reps = int(os.environ.get("KREPS", "1"))
            full = {"conv12", "conv3", "conv4", "xw1", "lstm", "head"} <= phases
            if not full:
                nc.any.memset(x3s[:, :, :], 0.0)
                nc.any.memset(x4s[:, :, :], 0.0)
                nc.any.memset(encs[:, :, :], 0.0)
                nc.any.memset(xw1s[:, :, :], 0.0)
                nc.any.memset(h2hist[:, :, :], 0.0)

            for _rep in range(reps):
                # ---- conv1 + conv2, per 4-frame group ----
                if "conv12" in phases:
                    for g in range(F // 4):
                        ps1 = pspool.tile([128, N1], F32, tag="ps")
                        for j in range(4):
                            f = 4 * g + j
                            ict = icpool.tile([48, N1], BF16, tag="ic")
                            nc.sync.dma_start(out=ict[:, :], in_=ic1p[:, f, :])
                            for n0, nw in ((0, 512), (512, N1 - 512)):
                                nc.tensor.matmul(
                                    ps1[32 * j:32 * j + 32, n0:n0 + nw],
                                    w1s[:, :], ict[:, n0:n0 + nw],
                                    start=True, stop=True,
                                    tile_position=(0, 32 * j),
                                )
                        x2t = x2pool.tile([128, O1, O1], BF16, tag="x2")
                        x2f = x2t.rearrange("p a b -> p (a b)")
                        for n0, nw in ((0, 512), (512, N1 - 512)):
                            nc.scalar.activation(x2f[:, n0:n0 + nw],
                                                 ps1[:, n0:n0 + nw],
                                                 AF.Relu, bias=b1s[:, :])

                        ps2 = pspool.tile([128, 2, 512], F32, tag="ps")
                        for it, (kh, kw) in enumerate(TAPS):
                            for j in range(4):
                                half = j // 2
                                nc.tensor.matmul(
                                    ps2[64 * half:64 * half + 64, j % 2, 0:N2],
                                    w2s[32 * j:32 * j + 32, it, :],
                                    x2t[32 * j:32 * j + 32,
                                        kh:kh + 27:2, kw:kw + 27:2],
                                    start=(it == 0), stop=(it == 15),
                                    tile_position=(32 * j, 64 * half),
                                )
                        for k in range(2):
                            nc.scalar.activation(x3s[:, 2 * g + k, :],
                                                 ps2[:, k, 0:N2],
                                                 AF.Relu, bias=b2s[:, :])

                # ---- conv3: 16 taps K=64, pairs batched 8 per matmul ----
                if "conv3" in phases:
                    x3v = x3s.rearrange("p n (r s) -> p n r s", r=O2)
                    x4q = x4s.rearrange("p (q i l) x -> p q i l x", q=8, i=4, l=4)
                    for q in range(8):
                        ps3 = pspool.tile([128, 2, 512], F32, tag="ps")
                        for it, (kh, kw) in enumerate(TAPS):
                            for h in range(2):
                                nc.tensor.matmul(
                                    ps3[:, h, 0:8 * N3],
                                    w3s[64 * h:64 * h + 64, it, :],
                                    x3v[64 * h:64 * h + 64, 8 * q:8 * q + 8,
                                        kh:kh + 11:2, kw:kw + 11:2],
                                    start=(it == 0), stop=(it == 15),
                                    tile_position=(64 * h, 0),
                                )
                        for h in range(2):
                            nc.scalar.activation(
                                x4q[:, q, :, 2 * h:2 * h + 2, :],
                                ps3[:, h, 0:8 * N3].rearrange(
                                    "p (n x) -> p n x", x=N3),
                                AF.Relu, bias=b3s[:, :])

                # ---- conv4: 16 taps K=128, all frames batched ----
                if "conv4" in phases:
                    x4v = x4s.rearrange("p f (r s) -> p f r s", r=O3)
                    ps4 = pspool.tile([128, 2, 512], F32, tag="ps")
                    for it, (kh, kw) in enumerate(TAPS):
                        for mh in range(2):
                            nc.tensor.matmul(
                                ps4[:, mh, :],
                                w4s[:, it, 128 * mh:128 * mh + 128],
                                x4v[:, :, kh:kh + 3:2, kw:kw + 3:2],
                                start=(it == 0), stop=(it == 15),
                            )
                    ps4v = ps4.rearrange("p m (f x) -> p m f x", x=N4)
                    for p in range(4):
                        for mh in range(2):
                            nc.scalar.activation(encs[:, 2 * p + mh, :],
                                                 ps4v[:, mh, :, p],
                                                 AF.Relu, bias=b4s[:, mh:mh + 1])

                # ---- x-projection for LSTM layer 1, batched over (b,t) ----
                if "xw1" in phases:
                    for m in range(16):
                        psx = psspool.tile([128, F], F32, tag="pss")
                        for kt in range(8):
                            nc.tensor.matmul(psx[:, :],
                                             wx1s[:, kt, 128 * m:128 * m + 128],
                                             encs[:, kt, :],
                                             start=(kt == 0), stop=False)
                        nc.tensor.matmul(psx[:, :],
                                         wacts[:, 128 * m:128 * m + 128],
                                         acts[:, :],
                                         start=False, stop=True)
                        nc.scalar.activation(xw1s[:, m, :], psx[:, :],
                                             AF.Identity, bias=bl1s[:, m:m + 1])

                # ---- LSTM recurrence, 64 steps, batch=2 per core ----
                if "lstm" in phases:
                    h1 = st.tile([128, 8], BF16, tag="h1z")
                    c1 = st.tile([128, 8], F32, tag="c1z")
                    h2 = st.tile([128, 4], BF16, tag="h2z")
                    c2 = st.tile([128, 4], F32, tag="c2z")
                    nc.any.memset(h1[:, :], 0.0)
                    nc.any.memset(c1[:, :], 0.0)
                    nc.any.memset(h2[:, :], 0.0)
                    nc.any.memset(c2[:, :], 0.0)
                    xw1v = xw1s.rearrange("p m (b t) -> p m b t", b=BP)
                    hv = h2hist.rearrange("p k (b t) -> p k b t", b=BP)

                    for t in range(T):
                        pg1 = psspool.tile([128, 32], F32, tag="pss")
                        for m in range(16):
                            for kt in range(4):
                                nc.tensor.matmul(pg1[:, 2 * m:2 * m + 2],
                                                 wh1s[:, kt, 128 * m:128 * m + 128],
                                                 h1[:, 2 * kt:2 * kt + 2],
                                                 start=(kt == 0), stop=(kt == 3))
                        g1 = sc.tile([128, 32], F32, tag="g1")
                        nc.vector.tensor_add(g1[:, :], pg1[:, :], xw1v[:, :, :, t])
                        nc.scalar.activation(g1[:, 0:8], g1[:, 0:8], AF.Sigmoid)
                        nc.scalar.activation(g1[:, 8:16], g1[:, 8:16], AF.Tanh)
                        nc.scalar.activation(g1[:, 16:32], g1[:, 16:32], AF.Sigmoid)
                        t1 = sc.tile([128, 8], F32, tag="t1")
                        nc.vector.tensor_mul(t1[:, :], g1[:, 16:24], c1[:, :])
                        t2 = sc.tile([128, 8], F32, tag="t2")
                        nc.vector.tensor_mul(t2[:, :], g1[:, 0:8], g1[:, 8:16])
                        c1 = st.tile([128, 8], F32, tag="c1")
                        nc.vector.tensor_add(c1[:, :], t1[:, :], t2[:, :])
                        th1 = sc.tile([128, 8], F32, tag="th1")
                        nc.scalar.activation(th1[:, :], c1[:, :], AF.Tanh)
                        h1 = st.tile([128, 8], BF16, tag="h1")
                        nc.vector.tensor_mul(h1[:, :], g1[:, 24:32], th1[:, :])

                        pg2 = psspool.tile([128, 16], F32, tag="pss")
                        for m in range(8):
                            for kt in range(6):
                                rhs = (h1[:, 2 * kt:2 * kt + 2] if kt < 4
                                       else h2[:, 2 * (kt - 4):2 * (kt - 4) + 2])
                                nc.tensor.matmul(pg2[:, 2 * m:2 * m + 2],
                                                 wl2s[:, kt, 128 * m:128 * m + 128],
                                                 rhs,
                                                 start=(kt == 0), stop=(kt == 5))
                        g2 = sc.tile([128, 16], F32, tag="g2")
                        nc.vector.tensor_add(g2[:, :], pg2[:, :], bl2s[:, :])
                        nc.scalar.activation(g2[:, 0:4], g2[:, 0:4], AF.Sigmoid)
                        nc.scalar.activation(g2[:, 4:8], g2[:, 4:8], AF.Tanh)
                        nc.scalar.activation(g2[:, 8:16], g2[:, 8:16], AF.Sigmoid)
                        t3 = sc.tile([128, 4], F32, tag="t3")
                        nc.vector.tensor_mul(t3[:, :], g2[:, 8:12], c2[:, :])
                        t4 = sc.tile([128, 4], F32, tag="t4")
                        nc.vector.tensor_mul(t4[:, :], g2[:, 0:4], g2[:, 4:8])
                        c2 = st.tile([128, 4], F32, tag="c2")
                        nc.vector.tensor_add(c2[:, :], t3[:, :], t4[:, :])
                        th2 = sc.tile([128, 4], F32, tag="th2")
                        nc.scalar.activation(th2[:, :], c2[:, :], AF.Tanh)
                        h2 = st.tile([128, 4], BF16, tag="h2")
                        nc.vector.tensor_mul(h2[:, :], g2[:, 12:16], th2[:, :])
                        nc.vector.tensor_copy(
                            hv[:, :, :, t],
                            h2[:, :].rearrange("p (k b) -> p k b", k=2))

                # ---- dense head, batched over (b,t) ----
                if "head" in phases:
                    pd1 = psspool.tile([128, F], F32, tag="pss")
                    for kt in range(2):
                        nc.tensor.matmul(pd1[:, :], wd1s[:, kt, :],
                                         h2hist[:, kt, :],
                                         start=(kt == 0), stop=(kt == 1))
                    d1t = sc.tile([128, F], BF16, tag="d1t")
                    nc.scalar.activation(d1t[:, :], pd1[:, :], AF.Relu,
                                         bias=bd1s[:, :])
                    pd2 = psspool.tile([1, F], F32, tag="pss")
                    nc.tensor.matmul(pd2[:, :], wd2s[:, :], d1t[:, :],
                                     start=True, stop=True)
                    osb = sc.tile([1, F], F32, tag="osb")
                    nc.scalar.activation(osb[:, :], pd2[:, :], AF.Identity,
                                         bias=bd2s[:, :])
                else:
                    osb = sc.tile([1, F], F32, tag="osb")
                    nc.any.memset(osb[:, :], 0.0)
            nc.sync.dma_start(out=outp[:, :], in_=osb[:, :])

    nc.finalize()
    return nc


def _prep_host(inputs):
    bf = ml_dtypes.bfloat16
    frames = np.ascontiguousarray(inputs["frames"], dtype=np.float32)
    actions = np.ascontiguousarray(inputs["actions"], dtype=np.float32)

    shared = {}
    w1r = np.asarray(inputs["w1"], np.float32).reshape(2, 2, 2, 2, 3, 32)
    shared["w1p"] = np.ascontiguousarray(
        w1r.transpose(0, 2, 1, 3, 4, 5).reshape(48, 32)).astype(bf)
    shared["b1p"] = np.tile(np.asarray(inputs["b1"], np.float32), 4)[:, None].copy()
    shared["w2p"] = np.ascontiguousarray(
        np.asarray(inputs["w2"], np.float32).reshape(16, 32, 64).transpose(1, 0, 2)
    ).astype(bf)
    shared["b2p"] = np.tile(np.asarray(inputs["b2"], np.float32), 2)[:, None].copy()
    shared["w3p"] = np.ascontiguousarray(
        np.asarray(inputs["w3"], np.float32).reshape(16, 64, 128).transpose(1, 0, 2)
    ).astype(bf)
    shared["b3p"] = np.asarray(inputs["b3"], np.float32)[:, None].copy()
    shared["w4p"] = np.ascontiguousarray(
        np.asarray(inputs["w4"], np.float32).reshape(16, 128, 256).transpose(1, 0, 2)
    ).astype(bf)
    shared["b4p"] = np.ascontiguousarray(
        np.asarray(inputs["b4"], np.float32).reshape(2, 128).T)
    kl1 = np.asarray(inputs["k_l1"], np.float32)
    shared["wx1p"] = np.ascontiguousarray(
        kl1[0:1024].reshape(8, 128, 2048).transpose(1, 0, 2)).astype(bf)
    shared["wactp"] = np.ascontiguousarray(kl1[1024:1026]).astype(bf)
    shared["wh1p"] = np.ascontiguousarray(
        kl1[1026:1538].reshape(4, 128, 2048).transpose(1, 0, 2)).astype(bf)
    bl1 = np.asarray(inputs["b_l1"], np.float32).copy()
    bl1[1024:1536] += 1.0          # fold forget bias
    shared["bl1p"] = np.ascontiguousarray(bl1.reshape(16, 128).T)
    kl2 = np.asarray(inputs["k_l2"], np.float32)
    shared["wl2p"] = np.ascontiguousarray(
        kl2.reshape(6, 128, 1024).transpose(1, 0, 2)).astype(bf)
    bl2 = np.asarray(inputs["b_l2"], np.float32).copy()
    bl2[512:768] += 1.0
    shared["bl2p"] = np.ascontiguousarray(
        np.repeat(bl2.reshape(8, 128).T, 2, axis=1))
    shared["wd1p"] = np.ascontiguousarray(
        np.asarray(inputs["w_d1"], np.float32).reshape(2, 128, 128).transpose(1, 0, 2)
    ).astype(bf)
    shared["bd1p"] = np.asarray(inputs["b_d1"], np.float32)[:, None].copy()
    shared["wd2p"] = np.asarray(inputs["w_d2"], np.float32).astype(bf).copy()
    shared["bd2p"] = np.asarray(inputs["b_d2"], np.float32).reshape(1, 1).copy()

    in_maps = []
    for c in range(NCORES):
        fr = frames[BP * c:BP * c + BP].reshape(F, H, W, C)
        z = fr.reshape(F, 32, 2, 32, 2, 3).transpose(0, 1, 3, 2, 4, 5)
        z = np.ascontiguousarray(z.reshape(F, 32, 32, 12))
        taps = [z[:, kh:kh + O1, kw:kw + O1, :]
                for kh, kw in ((0, 0), (0, 1), (1, 0), (1, 1))]
        ic1 = np.stack(taps, 0)                      # [4, F, 31, 31, 12]
        ic1 = ic1.transpose(0, 4, 1, 2, 3)           # [4, 12, F, 31, 31]
        ic1 = np.ascontiguousarray(ic1.reshape(48, F, N1)).astype(bf)
        actT = np.ascontiguousarray(
            actions[BP * c:BP * c + BP].reshape(F, 2).T).astype(bf)
        m = {"ic1": ic1, "actT": actT}
        m.update(shared)
        in_maps.append(m)
    return in_maps


def kernel(**inputs):
    from concourse.bass_utils import run_bass_kernel_spmd
    if "nc" not in _CACHE:
        _CACHE["nc"] = _build_graph()
    nc = _CACHE["nc"]
    in_maps = _prep_host(inputs)
    res = run_bass_kernel_spmd(nc, in_maps, core_ids=list(range(NCORES)),
                               trace=False)
    outs = [res.results[c]["out"].reshape(BP, T, 1) for c in range(NCORES)]
    return np.concatenate(outs, axis=0)



# revision 38
# speedup vs baseline: 43.7072x; 43.7072x over previous
"""AnticipatorRNN Trainium2 kernel: conv encoder + 2-layer LSTM + dense head.

Data-parallel over batch B=16 across 8 NeuronCores (2 sequences/core).
All matmuls in bf16 (f32 psum accumulate). No collectives.

v2: t-major frame order + chunked tiles so the Tile scheduler can overlap
conv-encoder work with the LSTM recurrence; batched ic1 DMAs; LSTM gates
reordered (i,f,o,j) so each layer needs 2 activation calls instead of 3;
h2 written straight into its history buffer (no per-step copy).
"""

import os
import numpy as np
import ml_dtypes

import concourse.bass as bass
import concourse.mybir as mybir
from concourse import bacc
import concourse.tile as tile

F32 = mybir.dt.float32
BF16 = mybir.dt.bfloat16
F8 = mybir.dt.float8e4
AF = mybir.ActivationFunctionType
F8SCALE = 16.0     # wh1 pre-scaled by 16, h1 by 1/16 (power-of-2: exact)

B, T, H, W, C = 16, 64, 64, 64, 3
NCORES = 8
BP = B // NCORES          # sequences per core = 2
F = BP * T                # frames per core = 128
O1, O2, O3, O4 = 31, 14, 6, 2
N1, N2, N3, N4 = O1 * O1, O2 * O2, O3 * O3, O4 * O4   # 961, 196, 36, 4

ICH = 16                  # frames per ic1 DMA chunk
NIC = F // ICH            # 8
# conv4/xw1 chunk sizes (frames): small leading chunks shorten the pipeline
# fill before the LSTM can start; larger ones amortize instruction count
CHUNKS = [int(x) for x in os.environ.get("KCHUNKS", "32,32,32,32").split(",")]
assert sum(CHUNKS) == F and all(c % ICH == 0 for c in CHUNKS)
NCC = len(CHUNKS)
F0 = [sum(CHUNKS[:i]) for i in range(NCC)]          # frame offset per chunk
STEP0 = [f // BP for f in F0]                        # first lstm step per chunk
NSTEPS = [c // BP for c in CHUNKS]                   # lstm steps per chunk

_CACHE = {}
TAPS = [(a, b) for a in range(4) for b in range(4)]


def _build_graph(phases=None):
    if phases is None:
        phases = set((os.environ.get("KPHASES")
                      or "conv12,conv3,conv4,xw1,lstm,head").split(","))
    nc = bacc.Bacc(None, target_bir_lowering=False)

    ic1p = nc.declare_dram_parameter("ic1", [48, F, N1], BF16, isOutput=False)
    actp = nc.declare_dram_parameter("actT", [2, F], BF16, isOutput=False)
    w1p = nc.declare_dram_parameter("w1p", [48, 32], BF16, isOutput=False)
    b1p = nc.declare_dram_parameter("b1p", [128, 1], F32, isOutput=False)
    w2p = nc.declare_dram_parameter("w2p", [32, 16, 64], BF16, isOutput=False)
    b2p = nc.declare_dram_parameter("b2p", [128, 1], F32, isOutput=False)
    w3p = nc.declare_dram_parameter("w3p", [64, 16, 128], BF16, isOutput=False)
    b3p = nc.declare_dram_parameter("b3p", [128, 1], F32, isOutput=False)
    w4p = nc.declare_dram_parameter("w4p", [128, 16, 256], BF16, isOutput=False)
    b4p = nc.declare_dram_parameter("b4p", [128, 2], F32, isOutput=False)
    wx1p = nc.declare_dram_parameter("wx1p", [128, 8, 2048], BF16, isOutput=False)
    wactp = nc.declare_dram_parameter("wactp", [2, 2048], BF16, isOutput=False)
    wh1p = nc.declare_dram_parameter("wh1p", [128, 4, 2048], F8, isOutput=False)
    bl1p = nc.declare_dram_parameter("bl1p", [128, 16], F32, isOutput=False)
    wl2p = nc.declare_dram_parameter("wl2p", [128, 6, 1024], BF16, isOutput=False)
    bl2p = nc.declare_dram_parameter("bl2p", [128, 16], F32, isOutput=False)
    wd1p = nc.declare_dram_parameter("wd1p", [128, 2, 128], BF16, isOutput=False)
    bd1p = nc.declare_dram_parameter("bd1p", [128, 1], F32, isOutput=False)
    wd2p = nc.declare_dram_parameter("wd2p", [128, 1], BF16, isOutput=False)
    bd2p = nc.declare_dram_parameter("bd2p", [1, 1], F32, isOutput=False)
    outp = nc.declare_dram_parameter("out", [1, F], F32, isOutput=True)

    with tile.TileContext(nc) as tc:
        with (
            tc.tile_pool(name="const", bufs=1) as const,
            tc.tile_pool(name="ic", bufs=2) as icpool,
            tc.tile_pool(name="x2", bufs=4) as x2pool,
            tc.tile_pool(name="big", bufs=1) as big,
            tc.tile_pool(name="ps", bufs=2, space="PSUM") as pspool,
            tc.tile_pool(name="psx", bufs=2, space="PSUM") as psxpool,
            tc.tile_pool(name="psg", bufs=2, space="PSUM") as psgpool,
            tc.tile_pool(name="st", bufs=3) as st,
            tc.tile_pool(name="sc", bufs=4) as sc,
        ):
            # ---- load weights/constants into SBUF ----
            w1s = const.tile([48, 32], BF16)
            nc.sync.dma_start(out=w1s[:, :], in_=w1p[:, :])
            b1s = const.tile([128, 1], F32)
            nc.sync.dma_start(out=b1s[:, :], in_=b1p[:, :])
            w2s = const.tile([128, 16, 64], BF16)
            for j in range(4):
                nc.sync.dma_start(out=w2s[32 * j:32 * j + 32, :, :], in_=w2p[:, :, :])
            b2s = const.tile([128, 1], F32)
            nc.sync.dma_start(out=b2s[:, :], in_=b2p[:, :])
            w3s = const.tile([128, 16, 128], BF16)
            for h in range(2):
                nc.sync.dma_start(out=w3s[64 * h:64 * h + 64, :, :], in_=w3p[:, :, :])
            b3s = const.tile([128, 1], F32)
            nc.sync.dma_start(out=b3s[:, :], in_=b3p[:, :])
            w4s = const.tile([128, 16, 256], BF16)
            nc.sync.dma_start(out=w4s[:, :, :], in_=w4p[:, :, :])
            b4s = const.tile([128, 2], F32)
            nc.sync.dma_start(out=b4s[:, :], in_=b4p[:, :])
            wx1s = const.tile([128, 8, 2048], BF16)
            nc.sync.dma_start(out=wx1s[:, :, :], in_=wx1p[:, :, :])
            wacts = const.tile([2, 2048], BF16)
            nc.sync.dma_start(out=wacts[:, :], in_=wactp[:, :])
            wh1s = const.tile([128, 4, 2048], F8)
            nc.sync.dma_start(out=wh1s[:, :, :], in_=wh1p[:, :, :])
            bl1s = const.tile([128, 16], F32)
            nc.sync.dma_start(out=bl1s[:, :], in_=bl1p[:, :])
            wl2s = const.tile([128, 6, 1024], BF16)
            nc.sync.dma_start(out=wl2s[:, :, :], in_=wl2p[:, :, :])
            bl2s = const.tile([128, 16], F32)
            nc.sync.dma_start(out=bl2s[:, :], in_=bl2p[:, :])
            wd1s = const.tile([128, 2, 128], BF16)
            nc.sync.dma_start(out=wd1s[:, :, :], in_=wd1p[:, :, :])
            bd1s = const.tile([128, 1], F32)
            nc.sync.dma_start(out=bd1s[:, :], in_=bd1p[:, :])
            wd2s = const.tile([128, 1], BF16)
            nc.sync.dma_start(out=wd2s[:, :], in_=wd2p[:, :])
            bd2s = const.tile([1, 1], F32)
            nc.sync.dma_start(out=bd2s[:, :], in_=bd2p[:, :])
            acts = const.tile([2, F], BF16)
            nc.sync.dma_start(out=acts[:, :], in_=actp[:, :])

            # persistent activations, chunked along the frame axis so the
            # scheduler can start downstream phases before upstream finish
            x3c = [big.tile([128, ICH // 2, N2], BF16, name=f"x3c{i}")
                   for i in range(NIC)]
            x4c = [big.tile([128, ICH, N3], BF16, name=f"x4c{i}")
                   for i in range(NIC)]
            encc = [big.tile([128, 8, CHUNKS[i]], BF16, name=f"encc{i}")
                    for i in range(NCC)]
            xw1c = [big.tile([128, 16, CHUNKS[i]], F32, name=f"xw1c{i}")
                    for i in range(NCC)]
            h2hist = big.tile([128, 2, F], BF16)    # [row, ktile, (t,b)]

            reps = int(os.environ.get("KREPS", "1"))
            hv = h2hist.rearrange("p k (t b) -> p k t b", t=T)
            icbox = {}
            S = {}

            def emit_ic_dma(ch):
                ict = icpool.tile([48, ICH, N1], BF16, tag="ic")
                nc.sync.dma_start(out=ict[:, :, :],
                                  in_=ic1p[:, ICH * ch:ICH * ch + ICH, :])
                icbox[ch] = ict

            def emit_conv12_group(ch, gl):
                ict = icbox[ch]
                ps1 = pspool.tile([128, N1], F32, tag="ps")
                for j in range(4):
                    fl = 4 * gl + j
                    for n0, nw in ((0, 512), (512, N1 - 512)):
                        nc.tensor.matmul(
                            ps1[32 * j:32 * j + 32, n0:n0 + nw],
                            w1s[:, :], ict[:, fl, n0:n0 + nw],
                            start=True, stop=True,
                            tile_position=(0, 32 * j),
                        )
                x2t = x2pool.tile([128, O1, O1], BF16, tag="x2")
                x2f = x2t.rearrange("p a b -> p (a b)")
                for n0, nw in ((0, 512), (512, N1 - 512)):
                    nc.scalar.activation(x2f[:, n0:n0 + nw],
                                         ps1[:, n0:n0 + nw],
                                         AF.Relu, bias=b1s[:, :])
                ps2 = pspool.tile([128, 2, 512], F32, tag="ps")
                for it, (kh, kw) in enumerate(TAPS):
                    for j in range(4):
                        half = j // 2
                        nc.tensor.matmul(
                            ps2[64 * half:64 * half + 64, j % 2, 0:N2],
                            w2s[32 * j:32 * j + 32, it, :],
                            x2t[32 * j:32 * j + 32,
                                kh:kh + 27:2, kw:kw + 27:2],
                            start=(it == 0), stop=(it == 15),
                            tile_position=(32 * j, 64 * half),
                        )
                for k in range(2):
                    nc.scalar.activation(x3c[ch][:, 2 * gl + k, :],
                                         ps2[:, k, 0:N2],
                                         AF.Relu, bias=b2s[:, :])

            def emit_conv3(q):
                x3v = x3c[q].rearrange("p n (r s) -> p n r s", r=O2)
                x4q = x4c[q].rearrange("p (i l) x -> p i l x", i=4)
                ps3 = pspool.tile([128, 2, 512], F32, tag="ps")
                for it, (kh, kw) in enumerate(TAPS):
                    for h in range(2):
                        nc.tensor.matmul(
                            ps3[:, h, 0:8 * N3],
                            w3s[64 * h:64 * h + 64, it, :],
                            x3v[64 * h:64 * h + 64, :,
                                kh:kh + 11:2, kw:kw + 11:2],
                            start=(it == 0), stop=(it == 15),
                            tile_position=(64 * h, 0),
                        )
                for h in range(2):
                    nc.scalar.activation(
                        x4q[:, :, 2 * h:2 * h + 2, :],
                        ps3[:, h, 0:8 * N3].rearrange("p (n x) -> p n x", x=N3),
                        AF.Relu, bias=b3s[:, :])

            def emit_conv4(cc):
                # one (mh, half) accumulation group at a time: groups share a
                # PSUM bank, and an interleaved start would re-mark another
                # group's in-flight partials as pending-zero
                cpi = CHUNKS[cc] // ICH
                ps4 = pspool.tile([128, 2, CHUNKS[cc] * N4], F32, tag="ps")
                for mh in range(2):
                    for half in range(cpi):
                        x4v = x4c[F0[cc] // ICH + half].rearrange(
                            "p f (r s) -> p f r s", r=O3)
                        for it, (kh, kw) in enumerate(TAPS):
                            nc.tensor.matmul(
                                ps4[:, mh,
                                    ICH * N4 * half:ICH * N4 * half + ICH * N4],
                                w4s[:, it, 128 * mh:128 * mh + 128],
                                x4v[:, :, kh:kh + 3:2, kw:kw + 3:2],
                                start=(it == 0), stop=(it == 15),
                            )
                ps4v = ps4.rearrange("p m (f x) -> p m f x", x=N4)
                for p in range(4):
                    for mh in range(2):
                        nc.scalar.activation(encc[cc][:, 2 * p + mh, :],
                                             ps4v[:, mh, :, p],
                                             AF.Relu, bias=b4s[:, mh:mh + 1])

            def emit_xw1(cc, m0, m1):
                for m in range(m0, m1):
                    psx = psxpool.tile([128, CHUNKS[cc]], F32, tag="psx")
                    for kt in range(8):
                        nc.tensor.matmul(psx[:, :],
                                         wx1s[:, kt, 128 * m:128 * m + 128],
                                         encc[cc][:, kt, :],
                                         start=(kt == 0), stop=False)
                    nc.tensor.matmul(psx[:, :],
                                     wacts[:, 128 * m:128 * m + 128],
                                     acts[:, F0[cc]:F0[cc] + CHUNKS[cc]],
                                     start=False, stop=True)
                    nc.scalar.activation(xw1c[cc][:, m, :], psx[:, :],
                                         AF.Identity, bias=bl1s[:, m:m + 1])

            def emit_lstm_init():
                h1 = st.tile([128, 8], BF16, tag="h1z")
                h1f8 = st.tile([128, 8], F8, tag="h1f8z")
                c1 = st.tile([128, 8], F32, tag="c1z")
                h2z = st.tile([128, 4], BF16, tag="h2z")
                c2 = st.tile([128, 4], F32, tag="c2z")
                nc.any.memset(h1[:, :], 0.0)
                nc.any.memset(h1f8[:, :], 0.0)
                nc.any.memset(c1[:, :], 0.0)
                nc.any.memset(h2z[:, :], 0.0)
                nc.any.memset(c2[:, :], 0.0)
                S.update(h1=h1, h1f8=h1f8, c1=c1, h2z=h2z, c2=c2)

            def emit_lstm_step(t):
                # gate order (host-permuted): i, f, o, j
                h1, c1 = S["h1"], S["c1"]
                cc = max(i for i in range(NCC) if STEP0[i] <= t)
                xw1v = xw1c[cc].rearrange("p m (t b) -> p m t b",
                                          t=NSTEPS[cc])
                tl = t - STEP0[cc]
                h1f8 = S["h1f8"]
                pg1 = psgpool.tile([128, 32], F32, tag="pg")
                for m in range(16):
                    for kt in range(4):
                        nc.tensor.matmul(pg1[:, 2 * m:2 * m + 2],
                                         wh1s[:, kt, 128 * m:128 * m + 128],
                                         h1f8[:, 2 * kt:2 * kt + 2],
                                         start=(kt == 0), stop=(kt == 3))
                g1 = sc.tile([128, 32], F32, tag="g1")
                nc.vector.tensor_add(g1[:, :], pg1[:, :], xw1v[:, :, tl, :])
                nc.scalar.activation(g1[:, 0:24], g1[:, 0:24], AF.Sigmoid)
                nc.scalar.activation(g1[:, 24:32], g1[:, 24:32], AF.Tanh)
                t1 = sc.tile([128, 8], F32, tag="t1")
                nc.vector.tensor_mul(t1[:, :], g1[:, 8:16], c1[:, :])
                t2 = sc.tile([128, 8], F32, tag="t2")
                nc.vector.tensor_mul(t2[:, :], g1[:, 0:8], g1[:, 24:32])
                c1 = st.tile([128, 8], F32, tag="c1")
                nc.vector.tensor_add(c1[:, :], t1[:, :], t2[:, :])
                th1 = sc.tile([128, 8], F32, tag="th1")
                nc.scalar.activation(th1[:, :], c1[:, :], AF.Tanh)
                h1 = st.tile([128, 8], BF16, tag="h1")
                nc.vector.tensor_mul(h1[:, :], g1[:, 16:24], th1[:, :])
                h1f8 = st.tile([128, 8], F8, tag="h1f8")
                nc.scalar.activation(h1f8[:, :], h1[:, :], AF.Identity,
                                     scale=1.0 / F8SCALE)
                S.update(h1=h1, h1f8=h1f8, c1=c1)

                c2 = S["c2"]
                pg2 = psgpool.tile([128, 16], F32, tag="pg")
                for m in range(8):
                    for kt in range(6):
                        if kt < 4:
                            rhs = h1[:, 2 * kt:2 * kt + 2]
                        elif t == 0:
                            rhs = S["h2z"][:, 2 * (kt - 4):2 * (kt - 4) + 2]
                        else:
                            rhs = hv[:, kt - 4, t - 1, :]
                        nc.tensor.matmul(pg2[:, 2 * m:2 * m + 2],
                                         wl2s[:, kt, 128 * m:128 * m + 128],
                                         rhs,
                                         start=(kt == 0), stop=(kt == 5))
                g2 = sc.tile([128, 16], F32, tag="g2")
                nc.vector.tensor_add(g2[:, :], pg2[:, :], bl2s[:, :])
                nc.scalar.activation(g2[:, 0:12], g2[:, 0:12], AF.Sigmoid)
                nc.scalar.activation(g2[:, 12:16], g2[:, 12:16], AF.Tanh)
                t3 = sc.tile([128, 4], F32, tag="t3")
                nc.vector.tensor_mul(t3[:, :], g2[:, 4:8], c2[:, :])
                t4 = sc.tile([128, 4], F32, tag="t4")
                nc.vector.tensor_mul(t4[:, :], g2[:, 0:4], g2[:, 12:16])
                c2 = st.tile([128, 4], F32, tag="c2")
                nc.vector.tensor_add(c2[:, :], t3[:, :], t4[:, :])
                th2 = sc.tile([128, 4], F32, tag="th2")
                nc.scalar.activation(th2[:, :], c2[:, :], AF.Tanh)
                nc.vector.tensor_mul(hv[:, :, t, :], g2[:, 8:12], th2[:, :])
                S.update(c2=c2)

            def conv_units(cc):
                units = []
                ics = range(F0[cc] // ICH, (F0[cc] + CHUNKS[cc]) // ICH)
                for ch in ics:
                    units.append(lambda ch=ch: emit_ic_dma(ch))
                    for gl in range(ICH // 4):
                        units.append(
                            lambda ch=ch, gl=gl: emit_conv12_group(ch, gl))
                for ch in ics:
                    units.append(lambda ch=ch: emit_conv3(ch))
                units.append(lambda: emit_conv4(cc))
                units.append(lambda: emit_xw1(cc, 0, 8))
                units.append(lambda: emit_xw1(cc, 8, 16))
                return units

            for _rep in range(reps):
                # software pipeline: conv chunk cc+1 is emitted interleaved
                # with the LSTM steps consuming chunk cc
                emit_lstm_init()
                for u in conv_units(0):
                    u()
                for cc in range(NCC):
                    units = conv_units(cc + 1) if cc + 1 < NCC else []
                    k = 0
                    for i in range(NSTEPS[cc]):
                        emit_lstm_step(STEP0[cc] + i)
                        while k < (i + 1) * len(units) // NSTEPS[cc]:
                            units[k]()
                            k += 1

                # ---- dense head, batched over (t,b) ----
                pd1 = psxpool.tile([128, F], F32, tag="psx")
                for kt in range(2):
                    nc.tensor.matmul(pd1[:, :], wd1s[:, kt, :],
                                     h2hist[:, kt, :],
                                     start=(kt == 0), stop=(kt == 1))
                d1t = sc.tile([128, F], BF16, tag="d1t")
                nc.scalar.activation(d1t[:, :], pd1[:, :], AF.Relu,
                                     bias=bd1s[:, :])
                pd2 = psxpool.tile([1, F], F32, tag="psx")
                nc.tensor.matmul(pd2[:, :], wd2s[:, :], d1t[:, :],
                                 start=True, stop=True)
                osb = sc.tile([1, F], F32, tag="osb")
                nc.scalar.activation(osb[:, :], pd2[:, :], AF.Identity,
                                     bias=bd2s[:, :])
            nc.sync.dma_start(out=outp[:, :], in_=osb[:, :])

    nc.finalize()
    return nc


def _gate_perm(hidden):
    """Column permutation taking TF gate order (i, j, f, o) to (i, f, o, j)."""
    h = hidden
    return np.concatenate([
        np.arange(0, h),              # i
        np.arange(2 * h, 3 * h),      # f
        np.arange(3 * h, 4 * h),      # o
        np.arange(h, 2 * h),          # j
    ])


def _prep_host(inputs):
    bf = ml_dtypes.bfloat16
    frames = np.ascontiguousarray(inputs["frames"], dtype=np.float32)
    actions = np.ascontiguousarray(inputs["actions"], dtype=np.float32)

    shared = {}
    w1r = np.asarray(inputs["w1"], np.float32).reshape(2, 2, 2, 2, 3, 32)
    shared["w1p"] = np.ascontiguousarray(
        w1r.transpose(0, 2, 1, 3, 4, 5).reshape(48, 32)).astype(bf)
    shared["b1p"] = np.tile(np.asarray(inputs["b1"], np.float32), 4)[:, None].copy()
    shared["w2p"] = np.ascontiguousarray(
        np.asarray(inputs["w2"], np.float32).reshape(16, 32, 64).transpose(1, 0, 2)
    ).astype(bf)
    shared["b2p"] = np.tile(np.asarray(inputs["b2"], np.float32), 2)[:, None].copy()
    shared["w3p"] = np.ascontiguousarray(
        np.asarray(inputs["w3"], np.float32).reshape(16, 64, 128).transpose(1, 0, 2)
    ).astype(bf)
    shared["b3p"] = np.asarray(inputs["b3"], np.float32)[:, None].copy()
    shared["w4p"] = np.ascontiguousarray(
        np.asarray(inputs["w4"], np.float32).reshape(16, 128, 256).transpose(1, 0, 2)
    ).astype(bf)
    shared["b4p"] = np.ascontiguousarray(
        np.asarray(inputs["b4"], np.float32).reshape(2, 128).T)

    p1 = _gate_perm(512)
    kl1 = np.asarray(inputs["k_l1"], np.float32)[:, :]
    bl1 = np.asarray(inputs["b_l1"], np.float32).copy()
    bl1[1024:1536] += 1.0          # fold forget bias (original order)
    kl1 = kl1[:, p1]
    bl1 = bl1[p1]
    shared["wx1p"] = np.ascontiguousarray(
        kl1[0:1024].reshape(8, 128, 2048).transpose(1, 0, 2)).astype(bf)
    shared["wactp"] = np.ascontiguousarray(kl1[1024:1026]).astype(bf)
    shared["wh1p"] = np.ascontiguousarray(
        (kl1[1026:1538] * F8SCALE).reshape(4, 128, 2048).transpose(1, 0, 2)
    ).astype(ml_dtypes.float8_e4m3)
    shared["bl1p"] = np.ascontiguousarray(bl1.reshape(16, 128).T)

    p2 = _gate_perm(256)
    kl2 = np.asarray(inputs["k_l2"], np.float32)
    bl2 = np.asarray(inputs["b_l2"], np.float32).copy()
    bl2[512:768] += 1.0
    kl2 = kl2[:, p2]
    bl2 = bl2[p2]
    shared["wl2p"] = np.ascontiguousarray(
        kl2.reshape(6, 128, 1024).transpose(1, 0, 2)).astype(bf)
    shared["bl2p"] = np.ascontiguousarray(
        np.repeat(bl2.reshape(8, 128).T, 2, axis=1))
    shared["wd1p"] = np.ascontiguousarray(
        np.asarray(inputs["w_d1"], np.float32).reshape(2, 128, 128).transpose(1, 0, 2)
    ).astype(bf)
    shared["bd1p"] = np.asarray(inputs["b_d1"], np.float32)[:, None].copy()
    shared["wd2p"] = np.asarray(inputs["w_d2"], np.float32).astype(bf).copy()
    shared["bd2p"] = np.asarray(inputs["b_d2"], np.float32).reshape(1, 1).copy()

    in_maps = []
    for c in range(NCORES):
        # t-major frame order: local frame index f = t*BP + b
        fr = frames[BP * c:BP * c + BP].transpose(1, 0, 2, 3, 4).reshape(
            F, H, W, C)
        z = fr.reshape(F, 32, 2, 32, 2, 3).transpose(0, 1, 3, 2, 4, 5)
        z = np.ascontiguousarray(z.reshape(F, 32, 32, 12))
        taps = [z[:, kh:kh + O1, kw:kw + O1, :]
                for kh, kw in ((0, 0), (0, 1), (1, 0), (1, 1))]
        ic1 = np.stack(taps, 0)                      # [4, F, 31, 31, 12]
        ic1 = ic1.transpose(0, 4, 1, 2, 3)           # [4, 12, F, 31, 31]
        ic1 = np.ascontiguousarray(ic1.reshape(48, F, N1)).astype(bf)
        actT = np.ascontiguousarray(
            actions[BP * c:BP * c + BP].transpose(1, 0, 2).reshape(F, 2).T
        ).astype(bf)
        m = {"ic1": ic1, "actT": actT}
        m.update(shared)
        in_maps.append(m)
    return in_maps


def kernel(**inputs):
    from concourse.bass_utils import run_bass_kernel_spmd
    if "nc" not in _CACHE:
        _CACHE["nc"] = _build_graph()
    nc = _CACHE["nc"]
    in_maps = _prep_host(inputs)
    res = run_bass_kernel_spmd(nc, in_maps, core_ids=list(range(NCORES)),
                               trace=False)
    # out[1, F] with F = (t, b): reshape to [T, BP] then swap to [BP, T, 1]
    outs = [res.results[c]["out"].reshape(T, BP).T.reshape(BP, T, 1)
            for c in range(NCORES)]
    return np.concatenate(outs, axis=0)
